# revision 13
# baseline (speedup 1.0000x reference)
"""AdvancedLogoViT on 8 Trainium2 NeuronCores.

Strategy
--------
- Data-parallel: 4 images per core, weights replicated. No collectives.
- Activations live in transposed ("XT") layout: [E on partitions (6x128),
  tokens on free dim]. 4 images concatenated -> 788 token columns.
- All projection matmuls run in fp32r (full PE rate for moving dim >= 256,
  ~1.5e-4 matmul rel err measured on HW). Attention internals (scores, softmax
  numerator, attn@v) run in bf16.
- Weights are pre-transposed AND pre-tiled on the host into the exact SBUF
  panel layouts the PE wants, so every weight DMA is a contiguous read.
- LayerNorm over E (partition dim) uses a constant (1/768) matmul that both
  reduces and broadcasts: mu_bc[p, n] = sum_k x[k, n]/768 for all p.
- Softmax: scores are computed transposed (keys on partitions), exp on ACT,
  row-sums via ones-matmul, 1/sum applied to attn@v output via a PE broadcast
  of the sums and a DVE reciprocal+multiply.
- The logo-attention per-head bias is constant along the softmax axis, so it
  cancels exactly; it (and the geo/txt/col feature extraction) is skipped.
- Biases / LN affine params that are identically zero / one in the inputs are
  folded out at build time (they are structurally so in setup_inputs).
"""
import numpy as np
import ml_dtypes
from contextlib import ExitStack

import concourse.bass as bass
import concourse.tile as tile
from concourse import mybir
from concourse.bass_utils import run_bass_kernel_spmd

F32 = mybir.dt.float32
F32R = mybir.dt.float32r
BF16 = mybir.dt.bfloat16
AF = mybir.ActivationFunctionType
ALU = mybir.AluOpType

NCORES = 8
BIMG = 4            # images per core
E, H, HD, FF = 768, 12, 64, 3072
NP1 = 197           # tokens per image (cls + 196)
T = BIMG * NP1      # 788 token columns per core
KC = E // 128       # 6 E-chunks
HALF = T // 2       # 394
SCALE = 1.0 / 8.0   # 1/sqrt(64)


# ----------------------------------------------------------------- host prep

def _panels(W):
    """W [O, I] -> [O/128, 128(p=I%128), I/128, 128(m)] weight-stationary lhsT
    panels: pan[mc, p, j, m] = W[mc*128+m, j*128+p]."""
    O, I = W.shape
    return np.ascontiguousarray(
        W.reshape(O // 128, 128, I // 128, 128).transpose(0, 3, 2, 1))


def _rhsw(W):
    """W [O, I] -> [128(p), I/128, O] moving-operand layout:
    r[p, j, n] = W[n, j*128+p]."""
    O, I = W.shape
    return np.ascontiguousarray(W.T.reshape(I // 128, 128, O).transpose(1, 0, 2))


def _host_prep(inp):
    g = {k: np.asarray(v, np.float32) for k, v in inp.items()}
    L = g['mha_w'].shape[0]
    d = {}
    flags = {}

    # conv1: lhsT per kx, contraction (c,ky)=24, replicated at partition 32*img
    w1 = g['conv1_w']  # [192, 3, 8, 8]
    w1t = np.zeros((8, 128, 192), np.float32)
    for kx in range(8):
        blk = w1[:, :, :, kx].reshape(192, 24).T  # [24(c,ky), 192]
        for img in range(BIMG):
            w1t[kx, img * 32:img * 32 + 24, :] = blk
    d['w1p'] = w1t
    # conv2: [kk, pc, mc, p, m] = conv2_w[mc*128+m, pc*128+p, kk//2, kk%2]
    w2 = g['conv2_w']  # [384, 192, 2, 2]
    w2t = np.zeros((4, 2, 3, 128, 128), np.float32)
    for kk in range(4):
        blk = w2[:, :, kk // 2, kk % 2]  # [384, 192]
        for pc in range(2):
            pl = min(128, 192 - pc * 128)
            for mc in range(3):
                w2t[kk, pc, mc, :pl, :] = blk[mc * 128:(mc + 1) * 128,
                                              pc * 128:pc * 128 + pl].T
    d['w2p'] = w2t
    d['w3p'] = _panels(g['conv3_w'].reshape(768, 384))      # [6, 128, 3, 128]
    d['pb'] = np.ascontiguousarray(
        (g['pos_embed'][0, 1:, :] + g['conv3_b'][None, :]).T.reshape(KC, 128, 196))
    d['cls0'] = np.ascontiguousarray(
        (g['cls_token'][0, 0] + g['pos_embed'][0, 0]).reshape(KC, 128))
    flags['b1'] = np.any(g['conv1_b'] != 0)
    flags['b2'] = np.any(g['conv2_b'] != 0)
    d['b1'] = np.zeros((128, 2), np.float32)
    d['b1'][:, 0] = g['conv1_b'][:128]
    d['b1'][:64, 1] = g['conv1_b'][128:]
    d['b2'] = np.ascontiguousarray(g['conv2_b'].reshape(3, 128).T)

    # transformer weights
    mw = g['mha_w']  # [L, 2304, 768]
    d['wqk'] = np.stack([_panels(mw[l, :1536]) for l in range(L)])   # [L,12,128,6,128]
    d['wv'] = np.stack([_rhsw(mw[l, 1536:]) for l in range(L)])      # [L,128,6,768]
    d['wo'] = np.stack([_panels(g['mha_ow'][l]) for l in range(L)])  # [L,6,128,6,128]
    d['wf1'] = np.stack([_panels(g['ff1_w'][l]) for l in range(L)])  # [L,24,128,6,128]
    d['wf2'] = np.stack([_panels(g['ff2_w'][l]) for l in range(L)])  # [L,6,128,24,128]
    lw = g['lqkv_w']
    d['wlqk'] = np.stack([_panels(lw[l, :1536]) for l in range(L)])
    d['wlv'] = np.stack([_rhsw(lw[l, 1536:]) for l in range(L)])
    d['wlp'] = np.stack([_panels(g['lproj_w'][l]) for l in range(L)])

    qkb = g['mha_b'][:, :1536].copy()
    qkb[:, :768] *= SCALE
    d['qkb'] = np.ascontiguousarray(qkb.reshape(L, 12, 128).transpose(0, 2, 1))
    flags['qkb'] = np.any(qkb != 0)
    d['vb'] = np.ascontiguousarray(
        np.broadcast_to(g['mha_b'][:, 1536:][:, None, :], (L, 128, 768)))
    flags['vb'] = np.any(g['mha_b'][:, 1536:] != 0)
    for nm, src in (('ob', 'mha_ob'), ('f2b', 'ff2_b'), ('lpb', 'lproj_b')):
        d[nm] = np.ascontiguousarray(g[src].reshape(L, KC, 128).transpose(0, 2, 1))
        flags[nm] = np.any(g[src] != 0)
    d['f1b'] = np.ascontiguousarray(g['ff1_b'].reshape(L, 24, 128).transpose(0, 2, 1))
    flags['f1b'] = np.any(g['ff1_b'] != 0)
    for nm in ('ln1_w', 'ln1_b', 'ln2_w', 'ln2_b'):
        d[nm] = np.ascontiguousarray(g[nm].reshape(L, KC, 128).transpose(0, 2, 1))
    flags['ln1_w'] = np.any(g['ln1_w'] != 1)
    flags['ln1_b'] = np.any(g['ln1_b'] != 0)
    flags['ln2_w'] = np.any(g['ln2_w'] != 1)
    flags['ln2_b'] = np.any(g['ln2_b'] != 0)
    d['normw'] = np.ascontiguousarray(g['norm_w'].reshape(KC, 128))
    d['normb'] = np.ascontiguousarray(g['norm_b'].reshape(KC, 128))
    flags['normw'] = np.any(g['norm_w'] != 1)
    flags['normb'] = np.any(g['norm_b'] != 0)
    d['clsw'] = _rhsw(g['cls_w'])                  # [128, 6, 4]
    d['clsb'] = np.ascontiguousarray(np.broadcast_to(g['cls_b'][None, :], (BIMG, 4)))
    flags['clsb'] = np.any(g['cls_b'] != 0)

    d['inv768'] = np.full((128, 128), 1.0 / 768.0, np.float32)
    d['ones_col'] = np.ones((128, 1), ml_dtypes.bfloat16)
    d['ones_r64'] = np.ones((1, 64), ml_dtypes.bfloat16)
    return d, flags, L


# ------------------------------------------------------------------- builder

def _fix_waits(nc, cap=1):
    """This container's walrus accepts <=1 sync-wait per instruction; Tile can
    attach several. Hoist extras onto single-wait NoOps just before the inst."""
    n_new = 0
    for f in nc.m.functions:
        for bb in f.blocks:
            out, changed = [], False
            for inst in bb.instructions:
                si = inst.sync_info
                if si is not None and len(si.on_wait) > cap:
                    waits = list(si.on_wait)
                    for j, w in enumerate(waits[:-cap]):
                        out.append(mybir.InstNoOp(
                            name=f"{inst.name}-xw{j}",
                            sync_info=mybir.SyncInfo(on_wait=[w], on_update=[]),
                            bass_nofuse=True, engine=inst.engine))
                        n_new += 1
                    si.on_wait = waits[-cap:]
                    changed = True
                out.append(inst)
            if changed:
                bb.instructions[:] = out
    return n_new


def build(L, flags, debug_taps=False):
    nc = bass.Bass()
    D = {}

    def din(name, shape, dt=F32R):
        D[name] = nc.dram_tensor(name, list(shape), dt, kind="ExternalInput")
        return D[name]

    x_d = din('x', (BIMG, 3, 224, 224))
    din('w1p', (8, 128, 192)); din('w2p', (4, 2, 3, 128, 128))
    din('w3p', (KC, 128, 3, 128))
    din('pb', (KC, 128, 196), F32)
    din('cls0', (KC, 128))
    din('b1', (128, 2), F32); din('b2', (128, 3), F32)
    din('wqk', (L, 12, 128, KC, 128)); din('wv', (L, 128, KC, 768))
    din('wo', (L, KC, 128, KC, 128))
    din('wf1', (L, 24, 128, KC, 128)); din('wf2', (L, KC, 128, 24, 128))
    din('wlqk', (L, 12, 128, KC, 128)); din('wlv', (L, 128, KC, 768))
    din('wlp', (L, KC, 128, KC, 128))
    din('qkb', (L, 128, 12), F32); din('vb', (L, 128, 768), F32)
    din('ob', (L, 128, KC), F32); din('f1b', (L, 128, 24), F32)
    din('f2b', (L, 128, KC), F32); din('lpb', (L, 128, KC), F32)
    din('ln1_w', (L, 128, KC), F32); din('ln1_b', (L, 128, KC), F32)
    din('ln2_w', (L, 128, KC), F32); din('ln2_b', (L, 128, KC), F32)
    din('normw', (128, KC), F32); din('normb', (128, KC), F32)
    din('clsw', (128, KC, 4)); din('clsb', (BIMG, 4), F32)
    din('inv768', (128, 128)); din('ones_col', (128, 1), BF16)
    din('ones_r64', (1, 64), BF16)
    out_d = nc.dram_tensor('out', [BIMG, 4], F32, kind="ExternalOutput")
    taps = {}
    if debug_taps:
        dbg_d = nc.dram_tensor('dbg', [L + 1, 128, KC, T], F32R, kind="ExternalOutput")

    def tap(name, tile, shape):
        if not debug_taps:
            return
        t = nc.dram_tensor('tap_' + name, list(shape), F32R, kind="ExternalOutput")
        taps[name] = t
        if tile.dtype == BF16:
            nc.gpsimd.dma_start(out=t[...], in_=tile)
        else:
            nc.sync.dma_start(out=t[...], in_=tile)

    with tile.TileContext(nc) as tc, ExitStack() as ctx:
        P = {}
        P['const'] = ctx.enter_context(tc.tile_pool(name="const", bufs=1))
        P['resid'] = ctx.enter_context(tc.tile_pool(name="resid", bufs=2))
        P['ps1'] = ctx.enter_context(tc.tile_pool(name="ps1", bufs=4, space="PSUM"))
        P['ps2'] = ctx.enter_context(tc.tile_pool(name="ps2", bufs=1, space="PSUM"))

        conv_ctx = ExitStack()
        P['conv'] = conv_ctx.enter_context(tc.tile_pool(name="conv", bufs=1))
        inv768 = P['const'].tile([128, 128], F32R)
        nc.sync.dma_start(out=inv768, in_=D['inv768'][:, :])
        ones_col = P['const'].tile([128, 1], BF16)
        nc.sync.dma_start(out=ones_col, in_=D['ones_col'][:, :])
        ones_r64 = P['const'].tile([1, 64], BF16)
        nc.sync.dma_start(out=ones_r64, in_=D['ones_r64'][:, :])
        eps_t = P['const'].tile([128, 1], F32)
        nc.vector.memset(eps_t, 1e-5)

        # ---------------- conv patch embedding -> y [128, 6, 788] f32r ----
        y = P['resid'].tile([128, KC, T], F32R, tag="resid")
        cls_bc = D['cls0'].rearrange("j p -> p j")
        for img in range(BIMG):
            nc.sync.dma_start(
                out=y.rearrange("p j (i n) -> p j i n", n=NP1)[:, :, img, 0:1],
                in_=cls_bc[:, :, None])

        w1p = P['conv'].tile([128, 8, 192], F32R)
        nc.sync.dma_start(out=w1p, in_=D['w1p'].rearrange("k p m -> p k m"))
        w2p = P['conv'].tile([128, 4, 2, 3, 128], F32R)
        nc.sync.dma_start(out=w2p, in_=D['w2p'].rearrange("k c m p n -> p k c m n"))
        w3p = P['conv'].tile([128, KC, 3, 128], F32R)
        nc.sync.dma_start(out=w3p, in_=D['w3p'].rearrange("m p j n -> p m j n"))
        b1t = P['conv'].tile([128, 2], F32)
        nc.sync.dma_start(out=b1t, in_=D['b1'][:, :])
        b2t = P['conv'].tile([128, 3], F32)
        nc.sync.dma_start(out=b2t, in_=D['b2'][:, :])
        pbt = P['conv'].tile([128, KC, 196], F32)
        nc.sync.dma_start(out=pbt, in_=D['pb'].rearrange("j p n -> p j n"))

        for pair in range(2):
            # x_sb: partitions img_loc*32 + c*8 + ky ; free (py, x)
            x_sb = P['conv'].tile([128, 28, 224], F32R, tag="xin")
            for c in range(3):
                srcc = x_d[pair * 2:(pair + 1) * 2, c].rearrange(
                    "i (py ky) x -> i ky py x", ky=8)
                for ky in range(8):
                    nc.sync.dma_start(
                        out=x_sb.rearrange("(i k) py x -> i k py x", i=4)
                                [0:2, c * 8 + ky, :, :],
                        in_=srcc[:, ky, :, :])
            xg = x_sb.rearrange("p py (px k) -> p py px k", k=8)
            y1 = P['conv'].tile([128, 2, 28, 28], F32R, tag="y1")   # [c1<=128, img, py, px]
            y1b = P['conv'].tile([64, 2, 28, 28], F32R, tag="y1b")
            for oc, (obase, olen, ytile) in enumerate(
                    ((0, 128, y1), (128, 64, y1b))):
                for im in range(2):
                    for ph in range(2):  # py half
                        ps = P['ps1'].tile([olen, 392], F32, tag="ps1")
                        for kx in range(8):
                            nc.tensor.matmul(
                                ps,
                                w1p[im * 32:im * 32 + 24, kx, obase:obase + olen],
                                xg[im * 32:im * 32 + 24, ph * 14:(ph + 1) * 14, :, kx],
                                start=(kx == 0), stop=(kx == 7),
                                tile_position=(im * 32, 0))
                        nc.scalar.activation(
                            out=ytile[:, im, ph * 14:(ph + 1) * 14, :]
                                .rearrange("p a b -> p (a b)"),
                            in_=ps, func=AF.Gelu,
                            bias=(b1t[:olen, oc:oc + 1] if flags['b1'] else 0.0))
            # conv2: contraction (kk, c1) ; rhs strided from y1
            y2c = P['conv'].tile([128, 3, 2, 196], F32R, tag="y2c")  # [p, mc, img, n]
            for mc in range(3):
                ps = P['ps1'].tile([128, 2, 196], F32, tag="ps1")
                n_acc = 0
                for kk in range(4):
                    ky2, kx2 = kk // 2, kk % 2
                    for pc, (ytile, plen) in enumerate(((y1, 128), (y1b, 64))):
                        rhs = ytile.rearrange(
                            "p i (a ky) (b kx) -> p i ky kx a b", ky=2, kx=2)
                        rhs = rhs[:plen, :, ky2, kx2, :, :]
                        nc.tensor.matmul(
                            ps, w2p[:plen, kk, pc, mc, :], rhs,
                            start=(n_acc == 0), stop=(n_acc == 7))
                        n_acc += 1
                nc.scalar.activation(
                    out=y2c[:, mc, :, :].rearrange("p a b -> p (a b)"),
                    in_=ps.rearrange("p a b -> p (a b)"), func=AF.Gelu,
                    bias=(b2t[:, mc:mc + 1] if flags['b2'] else 0.0))
            # conv3 1x1 + pos embed -> y patch cols
            for j in range(KC):
                ps = P['ps1'].tile([128, 2, 196], F32, tag="ps1")
                for kc in range(3):
                    nc.tensor.matmul(ps, w3p[:, j, kc, :], y2c[:, kc, :, :],
                                     start=(kc == 0), stop=(kc == 2))
                for im in range(2):
                    img = pair * 2 + im
                    nc.vector.tensor_add(
                        y[:, j, img * NP1 + 1:img * NP1 + 197],
                        ps[:, im, :], pbt[:, j, :])

        conv_ctx.close()
        P['wp'] = ctx.enter_context(tc.tile_pool(name="wp", bufs=3))
        P['wv'] = ctx.enter_context(tc.tile_pool(name="wvp", bufs=1))
        P['w24'] = ctx.enter_context(tc.tile_pool(name="w24", bufs=2))
        P['qk'] = ctx.enter_context(tc.tile_pool(name="qk", bufs=1))
        P['v'] = ctx.enter_context(tc.tile_pool(name="v", bufs=1))
        P['pt'] = ctx.enter_context(tc.tile_pool(name="pt", bufs=8))
        P['small'] = ctx.enter_context(tc.tile_pool(name="small", bufs=2))
        P['stat'] = ctx.enter_context(tc.tile_pool(name="stat", bufs=1))
        P['relu'] = ctx.enter_context(tc.tile_pool(name="relu", bufs=24))
        P['scr'] = ctx.enter_context(tc.tile_pool(name="scr", bufs=1))

        if debug_taps:
            nc.sync.dma_start(out=dbg_d[0], in_=y)

        # ---------------- helpers ----------------------------------------
        def layernorm(src, dst, c0, cn, wt, bt, wnz, bnz):
            """dst[:, :, c0:c0+cn] = LN(src[:, :, c0:c0+cn]) over E."""
            ps_mu = P['ps1'].tile([128, cn], F32, tag="ps1")
            for j in range(KC):
                nc.tensor.matmul(ps_mu, inv768, src[:, j, c0:c0 + cn],
                                 start=(j == 0), stop=(j == KC - 1))
            ps_m2 = P['ps1'].tile([128, cn], F32, tag="ps1")
            for j in range(KC):
                xsq = P['stat'].tile([128, cn], F32R, tag="xsq", bufs=2)
                nc.vector.tensor_mul(xsq, src[:, j, c0:c0 + cn],
                                     src[:, j, c0:c0 + cn])
                nc.tensor.matmul(ps_m2, inv768, xsq,
                                 start=(j == 0), stop=(j == KC - 1))
            musq = P['stat'].tile([128, cn], F32, tag="musq")
            nc.scalar.activation(out=musq, in_=ps_mu, func=AF.Square)
            var = P['stat'].tile([128, cn], F32, tag="var")
            nc.vector.tensor_sub(var, ps_m2, musq)
            sd = P['stat'].tile([128, cn], F32, tag="sd")
            nc.scalar.activation(out=sd, in_=var, func=AF.Sqrt, bias=eps_t)
            rstd = P['stat'].tile([128, cn], F32, tag="rstd")
            nc.vector.reciprocal(rstd, sd)
            for j in range(KC):
                t1 = P['stat'].tile([128, cn], F32, tag="t1", bufs=2)
                nc.vector.tensor_sub(t1, src[:, j, c0:c0 + cn], ps_mu)
                if wnz:
                    nc.vector.scalar_tensor_tensor(
                        out=dst[:, j, c0:c0 + cn], in0=t1,
                        scalar=wt[:, j:j + 1], in1=rstd,
                        op0=ALU.mult, op1=ALU.mult)
                else:
                    nc.vector.tensor_mul(dst[:, j, c0:c0 + cn], t1, rstd)
                if bnz:
                    nc.vector.tensor_scalar_add(
                        dst[:, j, c0:c0 + cn], dst[:, j, c0:c0 + cn],
                        bt[:, j:j + 1])

        def proj_ws(wdram, l, n_mc, rhs_fn, epi_fn, kc=KC, wtag="wp6", pool='wp'):
            """Weight-stationary projection: out-chunk mc x accumulate kc."""
            for mc in range(n_mc):
                wt = P[pool].tile([128, kc, 128], F32R, tag=wtag)
                nc.sync.dma_start(out=wt, in_=wdram[l, mc].rearrange("p j m -> p j m"))
                for half in range(2):
                    rhs_list = rhs_fn(half)
                    ps = P['ps1'].tile([128] + list(rhs_list[0][1]), F32, tag="ps1")
                    for j, (rhs, _shape) in enumerate(rhs_list):
                        nc.tensor.matmul(ps, wt[:, j, :], rhs,
                                         start=(j == 0), stop=(j == kc - 1))
                    epi_fn(mc, half, ps)

        def attention(q_sb, k_sb, Vs, o_sb, col0, nq, mchunks):
            """One image's multi-head attention. Vs: list of (tile, m0, mlen)."""
            r_heads = {}
            pts = {}
            for h in range(H):
                hp, j = (h % 2) * 64, h // 2
                ps_r = P['ps1'].tile([1, nq], F32, tag="psr", bufs=2)
                pts[h] = []
                for ci, (m0, mlen) in enumerate(mchunks):
                    ps_st = P['ps1'].tile([mlen, nq], F32, tag="ps1")
                    nc.tensor.matmul(
                        ps_st,
                        k_sb[hp:hp + 64, j, col0 + m0:col0 + m0 + mlen],
                        q_sb[hp:hp + 64, j, col0:col0 + nq],
                        start=True, stop=True)
                    pt = P['pt'].tile([mlen, nq], BF16, tag="pt")
                    nc.scalar.activation(out=pt, in_=ps_st, func=AF.Exp)
                    nc.tensor.matmul(ps_r, ones_col[:mlen, :], pt,
                                     start=(ci == 0), stop=(ci == len(mchunks) - 1))
                    pts[h].append((pt, m0, mlen))
                r_s = P['small'].tile([1, nq], BF16, tag="rs")
                nc.scalar.activation(out=r_s, in_=ps_r, func=AF.Copy)
                r_heads[h] = r_s
                if h % 2 == 1:
                    ps_o = P['ps1'].tile([128, nq], F32, tag="ps1")
                    for hh in (h - 1, h):
                        colp = (hh % 2) * 64
                        for ci, (pt, m0, mlen) in enumerate(pts[hh]):
                            nc.tensor.matmul(
                                ps_o[colp:colp + 64, :],
                                Vs[ci][0][:, hh * 64:hh * 64 + 64], pt,
                                start=(ci == 0), stop=(ci == len(pts[hh]) - 1),
                                tile_position=(0, colp))
                    ps_R = P['ps1'].tile([128, nq], F32, tag="ps1")
                    for hh in (h - 1, h):
                        colp = (hh % 2) * 64
                        nc.tensor.matmul(ps_R[colp:colp + 64, :], ones_r64,
                                         r_heads[hh], start=True, stop=True,
                                         tile_position=(0, colp))
                    rinv = P['small'].tile([128, nq], F32, tag="rinv")
                    nc.vector.reciprocal(rinv, ps_R)
                    nc.vector.tensor_mul(
                        o_sb[:, h // 2, col0:col0 + nq], ps_o, rinv)

        def vproj(src, wv_t, bias_nz, bias_t, col0, nt, vtag):
            """Activation-stationary V projection for one image -> token-major."""
            out = []
            mchunks = [(0, 128), (128, nt - 128)]
            for ci, (m0, mlen) in enumerate(mchunks):
                vt = P['v'].tile([mlen, 768], BF16, tag=vtag, bufs=2)
                for hf in range(2):
                    ps = P['ps2'].tile([mlen, 384], F32, tag="ps2", bufs=2)
                    for j in range(KC):
                        nc.tensor.matmul(
                            ps,
                            src[:, j, col0 + m0:col0 + m0 + mlen],
                            wv_t[:, j, hf * 384:(hf + 1) * 384],
                            start=(j == 0), stop=(j == KC - 1))
                    sl = vt[:, hf * 384:(hf + 1) * 384]
                    if bias_nz:
                        nc.vector.tensor_add(sl, ps, bias_t[:mlen, hf * 384:(hf + 1) * 384])
                    else:
                        nc.vector.tensor_copy(sl, ps)
                out.append((vt, m0, mlen))
            return out

        # ---------------- transformer layers ------------------------------
        for l in range(L):
            # qk projection (weight-stationary, 12 out-chunks)
            q_sb = P['qk'].tile([128, KC, T], BF16, tag="q")
            k_sb = P['qk'].tile([128, KC, T], BF16, tag="k")
            qkb_t = None
            if flags['qkb']:
                qkb_t = P['small'].tile([128, 12], F32, tag="qkb")
                nc.sync.dma_start(out=qkb_t, in_=D['qkb'][l])

            def qk_epi(mc, half, ps):
                dst = (q_sb if mc < KC else k_sb)
                jj = mc % KC
                sl = dst[:, jj, half * HALF:(half + 1) * HALF]
                if mc < KC:
                    nc.scalar.activation(
                        out=sl, in_=ps, func=AF.Copy, scale=SCALE,
                        bias=(qkb_t[:, mc:mc + 1] if flags['qkb'] else 0.0))
                else:
                    if flags['qkb']:
                        nc.vector.tensor_scalar_add(sl, ps, qkb_t[:, mc:mc + 1])
                    else:
                        nc.vector.tensor_copy(sl, ps)

            proj_ws(D['wqk'], l, 12,
                    lambda half: [(y[:, j, half * HALF:(half + 1) * HALF], (HALF,))
                                  for j in range(KC)],
                    qk_epi)

            if l == 0:
                tap('q', q_sb, (128, KC, T)); tap('k', k_sb, (128, KC, T))

            # v projection per image
            wv_t = P['wv'].tile([128, KC, 768], F32R, tag="wv")
            nc.sync.dma_start(out=wv_t, in_=D['wv'][l])
            vb_t = None
            if flags['vb']:
                vb_t = P['small'].tile([128, 768], F32, tag="vb")
                nc.sync.dma_start(out=vb_t, in_=D['vb'][l])
            Vs = [vproj(y, wv_t, flags['vb'], vb_t, img * NP1, NP1, f"v{img}")
                  for img in range(BIMG)]

            # attention per image -> o_sb
            o_sb = P['scr'].tile([128, KC, T], F32R, tag="osb")
            for img in range(BIMG):
                attention(q_sb, k_sb, Vs[img], o_sb, img * NP1, NP1,
                          [(0, 128), (128, 69)])

            if l == 0:
                tap('osb', o_sb, (128, KC, T))
                for im in range(BIMG):
                    for ci, (vt, m0, mlen) in enumerate(Vs[im]):
                        tap(f'v{im}_{ci}', vt, (mlen, 768))

            # o projection + residual -> s1, then LN1 -> y1
            ob_t = None
            if flags['ob']:
                ob_t = P['small'].tile([128, KC], F32, tag="ob")
                nc.sync.dma_start(out=ob_t, in_=D['ob'][l])
            s1 = P['resid'].tile([128, KC, T], F32R, tag="scr1", bufs=1)

            def o_epi(mc, half, ps):
                sl = s1[:, mc, half * HALF:(half + 1) * HALF]
                nc.vector.tensor_add(sl, ps, y[:, mc, half * HALF:(half + 1) * HALF])
                if flags['ob']:
                    nc.vector.tensor_scalar_add(sl, sl, ob_t[:, mc:mc + 1])

            proj_ws(D['wo'], l, KC,
                    lambda half: [(o_sb[:, j, half * HALF:(half + 1) * HALF], (HALF,))
                                  for j in range(KC)],
                    o_epi)

            if l == 0:
                tap('s1', s1, (128, KC, T))
            ln1w = ln1b = None
            if flags['ln1_w']:
                ln1w = P['small'].tile([128, KC], F32, tag="lnw")
                nc.sync.dma_start(out=ln1w, in_=D['ln1_w'][l])
            if flags['ln1_b']:
                ln1b = P['small'].tile([128, KC], F32, tag="lnb")
                nc.sync.dma_start(out=ln1b, in_=D['ln1_b'][l])
            y1 = P['resid'].tile([128, KC, T], F32R, tag="resid")
            for half in range(2):
                layernorm(s1, y1, half * HALF, HALF, ln1w, ln1b,
                          flags['ln1_w'], flags['ln1_b'])

            if l == 0:
                tap('y1', y1, (128, KC, T))
            # FFN
            f1b_t = None
            if flags['f1b']:
                f1b_t = P['small'].tile([128, 24], F32, tag="f1b")
                nc.sync.dma_start(out=f1b_t, in_=D['f1b'][l])
            f2b_t = None
            if flags['f2b']:
                f2b_t = P['small'].tile([128, KC], F32, tag="f2b")
                nc.sync.dma_start(out=f2b_t, in_=D['f2b'][l])
            s2 = P['resid'].tile([128, KC, T], F32R, tag="scr1", bufs=1)
            for half in range(2):
                relus = []

                def f1_epi(mc, hf, ps, _relus=relus, _half=half):
                    rt = P['relu'].tile([128, HALF], F32R, tag="relu")
                    nc.scalar.activation(
                        out=rt, in_=ps, func=AF.Relu,
                        bias=(f1b_t[:, mc:mc + 1] if flags['f1b'] else 0.0))
                    _relus.append(rt)

                for mc in range(24):
                    wt = P['wp'].tile([128, KC, 128], F32R, tag="wp6")
                    nc.sync.dma_start(out=wt, in_=D['wf1'][l, mc])
                    ps = P['ps1'].tile([128, HALF], F32, tag="ps1")
                    for j in range(KC):
                        nc.tensor.matmul(
                            ps, wt[:, j, :],
                            y1[:, j, half * HALF:(half + 1) * HALF],
                            start=(j == 0), stop=(j == KC - 1))
                    f1_epi(mc, half, ps)
                for mc in range(KC):
                    ps = P['ps1'].tile([128, HALF], F32, tag="ps1")
                    for kh in range(2):
                        wt = P['w24'].tile([128, 12, 128], F32R, tag="wp24")
                        nc.sync.dma_start(
                            out=wt, in_=D['wf2'][l, mc, :, kh * 12:(kh + 1) * 12, :])
                        for jj in range(12):
                            j = kh * 12 + jj
                            nc.tensor.matmul(ps, wt[:, jj, :], relus[j],
                                             start=(j == 0), stop=(j == 23))
                    sl = s2[:, mc, half * HALF:(half + 1) * HALF]
                    nc.vector.tensor_add(
                        sl, ps, y1[:, mc, half * HALF:(half + 1) * HALF])
                    if flags['f2b']:
                        nc.vector.tensor_scalar_add(sl, sl, f2b_t[:, mc:mc + 1])

            if l == 0:
                tap('s2', s2, (128, KC, T))
            ln2w = ln2b = None
            if flags['ln2_w']:
                ln2w = P['small'].tile([128, KC], F32, tag="lnw")
                nc.sync.dma_start(out=ln2w, in_=D['ln2_w'][l])
            if flags['ln2_b']:
                ln2b = P['small'].tile([128, KC], F32, tag="lnb")
                nc.sync.dma_start(out=ln2b, in_=D['ln2_b'][l])
            y2 = P['resid'].tile([128, KC, T], F32R, tag="resid")
            for half in range(2):
                layernorm(s2, y2, half * HALF, HALF, ln2w, ln2b,
                          flags['ln2_w'], flags['ln2_b'])

            if l == 0:
                tap('y2', y2, (128, KC, T))
            # ---- logo attention on patch tokens (no residual, cls kept) --
            lq_sb = P['qk'].tile([128, KC, T], BF16, tag="q")
            lk_sb = P['qk'].tile([128, KC, T], BF16, tag="k")

            def patch_rhs(src, j, half):
                return src[:, j, :].rearrange("p (i n) -> p i n", n=NP1)[
                    :, half * 2:(half + 1) * 2, 1:197]

            def lqk_epi(mc, half, ps):
                dst = (lq_sb if mc < KC else lk_sb)
                jj = mc % KC
                sl = patch_rhs(dst, jj, half)
                if mc < KC:
                    nc.scalar.activation(out=sl, in_=ps, func=AF.Copy, scale=SCALE)
                else:
                    nc.vector.tensor_copy(sl, ps)

            proj_ws(D['wlqk'], l, 12,
                    lambda half: [(patch_rhs(y2, j, half), (2, 196))
                                  for j in range(KC)],
                    lqk_epi)

            wlv_t = P['wv'].tile([128, KC, 768], F32R, tag="wv")
            nc.sync.dma_start(out=wlv_t, in_=D['wlv'][l])
            LVs = [vproj(y2, wlv_t, False, None, img * NP1 + 1, 196, f"v{img}")
                   for img in range(BIMG)]

            lo_sb = P['scr'].tile([128, KC, T], F32R, tag="osb")
            for img in range(BIMG):
                attention(lq_sb, lk_sb, LVs[img], lo_sb, img * NP1 + 1, 196,
                          [(0, 128), (128, 68)])

            lpb_t = None
            if flags['lpb']:
                lpb_t = P['small'].tile([128, KC], F32, tag="ob")
                nc.sync.dma_start(out=lpb_t, in_=D['lpb'][l])

            def lp_epi(mc, half, ps):
                sl = patch_rhs(y2, mc, half)
                if flags['lpb']:
                    nc.vector.tensor_scalar_add(sl, ps, lpb_t[:, mc:mc + 1])
                else:
                    nc.vector.tensor_copy(sl, ps)

            proj_ws(D['wlp'], l, KC,
                    lambda half: [(patch_rhs(lo_sb, j, half), (2, 196))
                                  for j in range(KC)],
                    lp_epi)

            y = y2
            if debug_taps:
                nc.sync.dma_start(out=dbg_d[l + 1], in_=y)

        # ---------------- final LN + classifier ---------------------------
        cls_ap = y.rearrange("p j (i n) -> p j i n", n=NP1)[:, :, :, 0]  # [128,6,4]
        normw_t = P['small'].tile([128, KC], F32, tag="lnw")
        nc.sync.dma_start(out=normw_t, in_=D['normw'][:, :])
        normb_t = P['small'].tile([128, KC], F32, tag="lnb")
        nc.sync.dma_start(out=normb_t, in_=D['normb'][:, :])
        yf = P['small'].tile([128, KC, BIMG], F32R, tag="yf")
        # LN over E on 4 cls columns
        xsq = P['stat'].tile([128, KC, BIMG], F32R, tag="fsq")
        for j in range(KC):
            nc.vector.tensor_mul(xsq[:, j, :], cls_ap[:, j, :], cls_ap[:, j, :])
        ps_mu = P['ps1'].tile([128, BIMG], F32, tag="ps1")
        for j in range(KC):
            nc.tensor.matmul(ps_mu, inv768, cls_ap[:, j, :],
                             start=(j == 0), stop=(j == KC - 1))
        ps_m2 = P['ps1'].tile([128, BIMG], F32, tag="ps1")
        for j in range(KC):
            nc.tensor.matmul(ps_m2, inv768, xsq[:, j, :],
                             start=(j == 0), stop=(j == KC - 1))
        musq = P['stat'].tile([128, BIMG], F32, tag="fmusq")
        nc.scalar.activation(out=musq, in_=ps_mu, func=AF.Square)
        var = P['stat'].tile([128, BIMG], F32, tag="fvar")
        nc.vector.tensor_sub(var, ps_m2, musq)
        sd = P['stat'].tile([128, BIMG], F32, tag="fsd")
        nc.scalar.activation(out=sd, in_=var, func=AF.Sqrt, bias=eps_t)
        rstd = P['stat'].tile([128, BIMG], F32, tag="frstd")
        nc.vector.reciprocal(rstd, sd)
        for j in range(KC):
            t1 = P['stat'].tile([128, BIMG], F32, tag="ft1")
            nc.vector.tensor_sub(t1, cls_ap[:, j, :], ps_mu)
            if flags['normw']:
                nc.vector.scalar_tensor_tensor(
                    out=yf[:, j, :], in0=t1, scalar=normw_t[:, j:j + 1],
                    in1=rstd, op0=ALU.mult, op1=ALU.mult)
            else:
                nc.vector.tensor_mul(yf[:, j, :], t1, rstd)
            if flags['normb']:
                nc.vector.tensor_scalar_add(yf[:, j, :], yf[:, j, :],
                                            normb_t[:, j:j + 1])
        clsw_t = P['small'].tile([128, KC, 4], F32R, tag="clsw")
        nc.sync.dma_start(out=clsw_t, in_=D['clsw'][:, :, :])
        ps_c = P['ps1'].tile([BIMG, 4], F32, tag="ps1")
        for j in range(KC):
            nc.tensor.matmul(ps_c, yf[:, j, :], clsw_t[:, j, :],
                             start=(j == 0), stop=(j == KC - 1))
        out_sb = P['small'].tile([BIMG, 4], F32, tag="outsb")
        if flags['clsb']:
            clsb_t = P['small'].tile([BIMG, 4], F32, tag="clsb")
            nc.sync.dma_start(out=clsb_t, in_=D['clsb'][:, :])
            nc.vector.tensor_add(out_sb, ps_c, clsb_t)
        else:
            nc.vector.tensor_copy(out_sb, ps_c)
        nc.sync.dma_start(out=out_d[:, :], in_=out_sb)

    _fix_waits(nc)
    nc._tap_names = list(taps)
    return nc


# -------------------------------------------------------------------- entry

_BUILD_CACHE = {}


def kernel(**inputs):
    d, flags, L = _host_prep(inputs)
    key = (L, tuple(sorted(flags.items())))
    debug_taps = bool(globals().get('DEBUG_TAPS'))
    key = key + (debug_taps,)
    if key not in _BUILD_CACHE:
        _BUILD_CACHE[key] = build(L, flags, debug_taps=debug_taps)
    nc = _BUILD_CACHE[key]
    x = np.asarray(inputs['x'], np.float32)
    in_maps = []
    for core in range(NCORES):
        m = dict(d)
        m['x'] = np.ascontiguousarray(x[core * BIMG:(core + 1) * BIMG])
        in_maps.append(m)
    br = run_bass_kernel_spmd(nc, in_maps, list(range(NCORES)))
    out = np.concatenate([br.results[i]['out'] for i in range(NCORES)], axis=0)
    if debug_taps:
        globals()['LAST_DBG'] = [br.results[i]['dbg'] for i in range(NCORES)]
        globals()['LAST_TAPS'] = [
            {k: br.results[i]['tap_' + k] for k in nc._tap_names}
            for i in range(NCORES)]
    return out


# revision 17
# speedup vs baseline: 1.1018x; 1.1018x over previous
"""AdvancedLogoViT on 8 Trainium2 NeuronCores.

Strategy
--------
- Data-parallel: 4 images per core, weights replicated. No collectives.
- Activations live in transposed ("XT") layout: [E on partitions (6x128),
  tokens on free dim]. 4 images concatenated -> 788 token columns.
- All projection matmuls run in fp32r (full PE rate for moving dim >= 256,
  ~1.5e-4 matmul rel err measured on HW). Attention internals (scores, softmax
  numerator, attn@v) run in bf16.
- Weights are pre-transposed AND pre-tiled on the host into the exact SBUF
  panel layouts the PE wants, so every weight DMA is a contiguous read.
- LayerNorm over E (partition dim) uses a constant (1/768) matmul that both
  reduces and broadcasts: mu_bc[p, n] = sum_k x[k, n]/768 for all p.
- Softmax: scores are computed transposed (keys on partitions), exp on ACT,
  row-sums via ones-matmul, 1/sum applied to attn@v output via a PE broadcast
  of the sums and a DVE reciprocal+multiply.
- The logo-attention per-head bias is constant along the softmax axis, so it
  cancels exactly; it (and the geo/txt/col feature extraction) is skipped.
- Biases / LN affine params that are identically zero / one in the inputs are
  folded out at build time (they are structurally so in setup_inputs).
"""
import numpy as np
import ml_dtypes
from contextlib import ExitStack

import concourse.bass as bass
import concourse.tile as tile
from concourse import mybir
from concourse.bass_utils import run_bass_kernel_spmd

F32 = mybir.dt.float32
F32R = mybir.dt.float32r
BF16 = mybir.dt.bfloat16
AF = mybir.ActivationFunctionType
ALU = mybir.AluOpType

NCORES = 8
BIMG = 4            # images per core
E, H, HD, FF = 768, 12, 64, 3072
NP1 = 197           # tokens per image (cls + 196)
T = BIMG * NP1      # 788 token columns per core
KC = E // 128       # 6 E-chunks
HALF = T // 2       # 394
SCALE = 1.0 / 8.0   # 1/sqrt(64)


# ----------------------------------------------------------------- host prep

def _panels(W):
    """W [O, I] -> [O/128, 128(p=I%128), I/128, 128(m)] weight-stationary lhsT
    panels: pan[mc, p, j, m] = W[mc*128+m, j*128+p]."""
    O, I = W.shape
    return np.ascontiguousarray(
        W.reshape(O // 128, 128, I // 128, 128).transpose(0, 3, 2, 1))


def _rhsw(W):
    """W [O, I] -> [128(p), I/128, O] moving-operand layout:
    r[p, j, n] = W[n, j*128+p]."""
    O, I = W.shape
    return np.ascontiguousarray(W.T.reshape(I // 128, 128, O).transpose(1, 0, 2))


def _host_prep(inp):
    g = {k: np.asarray(v, np.float32) for k, v in inp.items()}
    L = g['mha_w'].shape[0]
    d = {}
    flags = {}

    # conv1: lhsT per kx, contraction (c,ky)=24, replicated at partition 32*img
    w1 = g['conv1_w']  # [192, 3, 8, 8]
    w1t = np.zeros((8, 128, 192), np.float32)
    for kx in range(8):
        blk = w1[:, :, :, kx].reshape(192, 24).T  # [24(c,ky), 192]
        for img in range(BIMG):
            w1t[kx, img * 32:img * 32 + 24, :] = blk
    d['w1p'] = w1t
    # conv2: [kk, pc, mc, p, m] = conv2_w[mc*128+m, pc*128+p, kk//2, kk%2]
    w2 = g['conv2_w']  # [384, 192, 2, 2]
    w2t = np.zeros((4, 2, 3, 128, 128), np.float32)
    for kk in range(4):
        blk = w2[:, :, kk // 2, kk % 2]  # [384, 192]
        for pc in range(2):
            pl = min(128, 192 - pc * 128)
            for mc in range(3):
                w2t[kk, pc, mc, :pl, :] = blk[mc * 128:(mc + 1) * 128,
                                              pc * 128:pc * 128 + pl].T
    d['w2p'] = w2t
    d['w3p'] = _panels(g['conv3_w'].reshape(768, 384))      # [6, 128, 3, 128]
    d['pb'] = np.ascontiguousarray(
        (g['pos_embed'][0, 1:, :] + g['conv3_b'][None, :]).T.reshape(KC, 128, 196))
    d['cls0'] = np.ascontiguousarray(
        (g['cls_token'][0, 0] + g['pos_embed'][0, 0]).reshape(KC, 128))
    flags['b1'] = np.any(g['conv1_b'] != 0)
    flags['b2'] = np.any(g['conv2_b'] != 0)
    d['b1'] = np.zeros((128, 2), np.float32)
    d['b1'][:, 0] = g['conv1_b'][:128]
    d['b1'][:64, 1] = g['conv1_b'][128:]
    d['b2'] = np.ascontiguousarray(g['conv2_b'].reshape(3, 128).T)

    # transformer weights
    mw = g['mha_w']  # [L, 2304, 768]
    d['wqk'] = np.stack([_panels(mw[l, :1536]) for l in range(L)])   # [L,12,128,6,128]
    d['wv'] = np.stack([_rhsw(mw[l, 1536:]) for l in range(L)])      # [L,128,6,768]
    d['wo'] = np.stack([_panels(g['mha_ow'][l]) for l in range(L)])  # [L,6,128,6,128]
    d['wf1'] = np.stack([_panels(g['ff1_w'][l]) for l in range(L)])  # [L,24,128,6,128]
    d['wf2'] = np.stack([_panels(g['ff2_w'][l]) for l in range(L)])  # [L,6,128,24,128]
    lw = g['lqkv_w']
    d['wlqk'] = np.stack([_panels(lw[l, :1536]) for l in range(L)])
    d['wlv'] = np.stack([_rhsw(lw[l, 1536:]) for l in range(L)])
    d['wlp'] = np.stack([_panels(g['lproj_w'][l]) for l in range(L)])

    qkb = g['mha_b'][:, :1536].copy()
    qkb[:, :768] *= SCALE
    d['qkb'] = np.ascontiguousarray(qkb.reshape(L, 12, 128).transpose(0, 2, 1))
    flags['qkb'] = np.any(qkb != 0)
    d['vb'] = np.ascontiguousarray(
        np.broadcast_to(g['mha_b'][:, 1536:][:, None, :], (L, 128, 768)))
    flags['vb'] = np.any(g['mha_b'][:, 1536:] != 0)
    for nm, src in (('ob', 'mha_ob'), ('f2b', 'ff2_b'), ('lpb', 'lproj_b')):
        d[nm] = np.ascontiguousarray(g[src].reshape(L, KC, 128).transpose(0, 2, 1))
        flags[nm] = np.any(g[src] != 0)
    d['f1b'] = np.ascontiguousarray(g['ff1_b'].reshape(L, 24, 128).transpose(0, 2, 1))
    flags['f1b'] = np.any(g['ff1_b'] != 0)
    for nm in ('ln1_w', 'ln1_b', 'ln2_w', 'ln2_b'):
        d[nm] = np.ascontiguousarray(g[nm].reshape(L, KC, 128).transpose(0, 2, 1))
    flags['ln1_w'] = np.any(g['ln1_w'] != 1)
    flags['ln1_b'] = np.any(g['ln1_b'] != 0)
    flags['ln2_w'] = np.any(g['ln2_w'] != 1)
    flags['ln2_b'] = np.any(g['ln2_b'] != 0)
    d['normw'] = np.ascontiguousarray(g['norm_w'].reshape(KC, 128))
    d['normb'] = np.ascontiguousarray(g['norm_b'].reshape(KC, 128))
    flags['normw'] = np.any(g['norm_w'] != 1)
    flags['normb'] = np.any(g['norm_b'] != 0)
    d['clsw'] = _rhsw(g['cls_w'])                  # [128, 6, 4]
    d['clsb'] = np.ascontiguousarray(np.broadcast_to(g['cls_b'][None, :], (BIMG, 4)))
    flags['clsb'] = np.any(g['cls_b'] != 0)

    d['inv768'] = np.full((128, 128), 1.0 / 768.0, np.float32)
    d['ones_col'] = np.ones((128, 1), ml_dtypes.bfloat16)
    d['ones_r64'] = np.ones((128, 64), ml_dtypes.bfloat16)
    return d, flags, L


# ------------------------------------------------------------------- builder

def _fix_waits(nc, cap=1):
    """This container's walrus accepts <=1 sync-wait per instruction; Tile can
    attach several. Hoist extras onto single-wait NoOps just before the inst."""
    n_new = 0
    for f in nc.m.functions:
        for bb in f.blocks:
            out, changed = [], False
            for inst in bb.instructions:
                si = inst.sync_info
                if si is not None and len(si.on_wait) > cap:
                    waits = list(si.on_wait)
                    for j, w in enumerate(waits[:-cap]):
                        out.append(mybir.InstNoOp(
                            name=f"{inst.name}-xw{j}",
                            sync_info=mybir.SyncInfo(on_wait=[w], on_update=[]),
                            bass_nofuse=True, engine=inst.engine))
                        n_new += 1
                    si.on_wait = waits[-cap:]
                    changed = True
                out.append(inst)
            if changed:
                bb.instructions[:] = out
    return n_new


def build(L, flags, debug_taps=False):
    nc = bass.Bass()
    D = {}

    def din(name, shape, dt=F32R):
        D[name] = nc.dram_tensor(name, list(shape), dt, kind="ExternalInput")
        return D[name]

    x_d = din('x', (BIMG, 3, 224, 224))
    din('w1p', (8, 128, 192)); din('w2p', (4, 2, 3, 128, 128))
    din('w3p', (KC, 128, 3, 128))
    din('pb', (KC, 128, 196), F32)
    din('cls0', (KC, 128))
    din('b1', (128, 2), F32); din('b2', (128, 3), F32)
    din('wqk', (L, 12, 128, KC, 128)); din('wv', (L, 128, KC, 768))
    din('wo', (L, KC, 128, KC, 128))
    din('wf1', (L, 24, 128, KC, 128)); din('wf2', (L, KC, 128, 24, 128))
    din('wlqk', (L, 12, 128, KC, 128)); din('wlv', (L, 128, KC, 768))
    din('wlp', (L, KC, 128, KC, 128))
    din('qkb', (L, 128, 12), F32); din('vb', (L, 128, 768), F32)
    din('ob', (L, 128, KC), F32); din('f1b', (L, 128, 24), F32)
    din('f2b', (L, 128, KC), F32); din('lpb', (L, 128, KC), F32)
    din('ln1_w', (L, 128, KC), F32); din('ln1_b', (L, 128, KC), F32)
    din('ln2_w', (L, 128, KC), F32); din('ln2_b', (L, 128, KC), F32)
    din('normw', (128, KC), F32); din('normb', (128, KC), F32)
    din('clsw', (128, KC, 4)); din('clsb', (BIMG, 4), F32)
    din('inv768', (128, 128)); din('ones_col', (128, 1), BF16)
    din('ones_r64', (128, 64), BF16)
    out_d = nc.dram_tensor('out', [BIMG, 4], F32, kind="ExternalOutput")
    taps = {}
    if debug_taps:
        dbg_d = nc.dram_tensor('dbg', [L + 1, 128, KC, T], F32R, kind="ExternalOutput")

    def tap(name, tile, shape):
        if not debug_taps:
            return
        t = nc.dram_tensor('tap_' + name, list(shape), F32R, kind="ExternalOutput")
        taps[name] = t
        if tile.dtype == BF16:
            nc.gpsimd.dma_start(out=t[...], in_=tile)
        else:
            nc.sync.dma_start(out=t[...], in_=tile)

    with tile.TileContext(nc) as tc, ExitStack() as ctx:
        P = {}
        P['const'] = ctx.enter_context(tc.tile_pool(name="const", bufs=1))
        P['resid'] = ctx.enter_context(tc.tile_pool(name="resid", bufs=2))
        P['ps1'] = ctx.enter_context(tc.tile_pool(name="ps1", bufs=5, space="PSUM"))
        P['ps2'] = ctx.enter_context(tc.tile_pool(name="ps2", bufs=1, space="PSUM"))

        conv_ctx = ExitStack()
        P['conv'] = conv_ctx.enter_context(tc.tile_pool(name="conv", bufs=1))
        inv768 = P['const'].tile([128, 128], F32R)
        nc.sync.dma_start(out=inv768, in_=D['inv768'][:, :])
        ones_col = P['const'].tile([128, 1], BF16)
        nc.sync.dma_start(out=ones_col, in_=D['ones_col'][:, :])
        ones_r64 = P['const'].tile([128, 64], BF16)
        nc.sync.dma_start(out=ones_r64, in_=D['ones_r64'][:, :])
        eps_t = P['const'].tile([128, 1], F32)
        nc.vector.memset(eps_t, 1e-5)

        # ---------------- conv patch embedding -> y [128, 6, 788] f32r ----
        y = P['resid'].tile([128, KC, T], F32R, tag="resid")
        cls_bc = D['cls0'].rearrange("j p -> p j")
        for img in range(BIMG):
            nc.sync.dma_start(
                out=y.rearrange("p j (i n) -> p j i n", n=NP1)[:, :, img, 0:1],
                in_=cls_bc[:, :, None])

        w1p = P['conv'].tile([128, 8, 192], F32R)
        nc.sync.dma_start(out=w1p, in_=D['w1p'].rearrange("k p m -> p k m"))
        w2p = P['conv'].tile([128, 4, 2, 3, 128], F32R)
        nc.sync.dma_start(out=w2p, in_=D['w2p'].rearrange("k c m p n -> p k c m n"))
        w3p = P['conv'].tile([128, KC, 3, 128], F32R)
        nc.sync.dma_start(out=w3p, in_=D['w3p'].rearrange("m p j n -> p m j n"))
        b1t = P['conv'].tile([128, 2], F32)
        nc.sync.dma_start(out=b1t, in_=D['b1'][:, :])
        b2t = P['conv'].tile([128, 3], F32)
        nc.sync.dma_start(out=b2t, in_=D['b2'][:, :])
        pbt = P['conv'].tile([128, KC, 196], F32)
        nc.sync.dma_start(out=pbt, in_=D['pb'].rearrange("j p n -> p j n"))

        for pair in range(2):
            # x_sb: partitions img_loc*32 + c*8 + ky ; free (py, x)
            x_sb = P['conv'].tile([128, 28, 224], F32R, tag="xin")
            for c in range(3):
                srcc = x_d[pair * 2:(pair + 1) * 2, c].rearrange(
                    "i (py ky) x -> i ky py x", ky=8)
                for ky in range(8):
                    nc.sync.dma_start(
                        out=x_sb.rearrange("(i k) py x -> i k py x", i=4)
                                [0:2, c * 8 + ky, :, :],
                        in_=srcc[:, ky, :, :])
            xg = x_sb.rearrange("p py (px k) -> p py px k", k=8)
            y1 = P['conv'].tile([128, 2, 28, 28], F32R, tag="y1")   # [c1<=128, img, py, px]
            y1b = P['conv'].tile([64, 2, 28, 28], F32R, tag="y1b")
            for oc, (obase, olen, ytile) in enumerate(
                    ((0, 128, y1), (128, 64, y1b))):
                for im in range(2):
                    for ph in range(2):  # py half
                        ps = P['ps1'].tile([olen, 392], F32, tag="ps1")
                        for kx in range(8):
                            nc.tensor.matmul(
                                ps,
                                w1p[im * 32:im * 32 + 24, kx, obase:obase + olen],
                                xg[im * 32:im * 32 + 24, ph * 14:(ph + 1) * 14, :, kx],
                                start=(kx == 0), stop=(kx == 7),
                                tile_position=(im * 32, 0))
                        nc.scalar.activation(
                            out=ytile[:, im, ph * 14:(ph + 1) * 14, :]
                                .rearrange("p a b -> p (a b)"),
                            in_=ps, func=AF.Gelu,
                            bias=(b1t[:olen, oc:oc + 1] if flags['b1'] else 0.0))
            # conv2: contraction (kk, c1) ; rhs strided from y1
            y2c = P['conv'].tile([128, 3, 2, 196], F32R, tag="y2c")  # [p, mc, img, n]
            for mc in range(3):
                ps = P['ps1'].tile([128, 2, 196], F32, tag="ps1")
                n_acc = 0
                for kk in range(4):
                    ky2, kx2 = kk // 2, kk % 2
                    for pc, (ytile, plen) in enumerate(((y1, 128), (y1b, 64))):
                        rhs = ytile.rearrange(
                            "p i (a ky) (b kx) -> p i ky kx a b", ky=2, kx=2)
                        rhs = rhs[:plen, :, ky2, kx2, :, :]
                        nc.tensor.matmul(
                            ps, w2p[:plen, kk, pc, mc, :], rhs,
                            start=(n_acc == 0), stop=(n_acc == 7))
                        n_acc += 1
                nc.scalar.activation(
                    out=y2c[:, mc, :, :].rearrange("p a b -> p (a b)"),
                    in_=ps.rearrange("p a b -> p (a b)"), func=AF.Gelu,
                    bias=(b2t[:, mc:mc + 1] if flags['b2'] else 0.0))
            # conv3 1x1 + pos embed -> y patch cols
            for j in range(KC):
                ps = P['ps1'].tile([128, 2, 196], F32, tag="ps1")
                for kc in range(3):
                    nc.tensor.matmul(ps, w3p[:, j, kc, :], y2c[:, kc, :, :],
                                     start=(kc == 0), stop=(kc == 2))
                for im in range(2):
                    img = pair * 2 + im
                    nc.vector.tensor_add(
                        y[:, j, img * NP1 + 1:img * NP1 + 197],
                        ps[:, im, :], pbt[:, j, :])

        conv_ctx.close()
        P['wp'] = ctx.enter_context(tc.tile_pool(name="wp", bufs=5))
        P['wv'] = ctx.enter_context(tc.tile_pool(name="wvp", bufs=1))
        P['w24'] = ctx.enter_context(tc.tile_pool(name="w24", bufs=2))
        P['qk'] = ctx.enter_context(tc.tile_pool(name="qk", bufs=1))
        P['v'] = ctx.enter_context(tc.tile_pool(name="v", bufs=1))
        P['pt'] = ctx.enter_context(tc.tile_pool(name="pt", bufs=8))
        P['small'] = ctx.enter_context(tc.tile_pool(name="small", bufs=2))
        P['stat'] = ctx.enter_context(tc.tile_pool(name="stat", bufs=1))
        P['relu'] = ctx.enter_context(tc.tile_pool(name="relu", bufs=24))
        P['scr'] = ctx.enter_context(tc.tile_pool(name="scr", bufs=1))

        if debug_taps:
            nc.sync.dma_start(out=dbg_d[0], in_=y)

        # ---------------- helpers ----------------------------------------
        def layernorm(src, dst, c0, cn, wt, bt, wnz, bnz):
            """dst[:, :, c0:c0+cn] = LN(src[:, :, c0:c0+cn]) over E."""
            ps_mu = P['ps1'].tile([128, cn], F32, tag="ps1")
            for j in range(KC):
                nc.tensor.matmul(ps_mu, inv768, src[:, j, c0:c0 + cn],
                                 start=(j == 0), stop=(j == KC - 1))
            ps_m2 = P['ps1'].tile([128, cn], F32, tag="ps1")
            for j in range(KC):
                xsq = P['stat'].tile([128, cn], F32R, tag="xsq", bufs=2)
                nc.vector.tensor_mul(xsq, src[:, j, c0:c0 + cn],
                                     src[:, j, c0:c0 + cn])
                nc.tensor.matmul(ps_m2, inv768, xsq,
                                 start=(j == 0), stop=(j == KC - 1))
            musq = P['stat'].tile([128, cn], F32, tag="musq")
            nc.scalar.activation(out=musq, in_=ps_mu, func=AF.Square)
            var = P['stat'].tile([128, cn], F32, tag="var")
            nc.vector.tensor_sub(var, ps_m2, musq)
            sd = P['stat'].tile([128, cn], F32, tag="sd")
            nc.scalar.activation(out=sd, in_=var, func=AF.Sqrt, bias=eps_t)
            rstd = P['stat'].tile([128, cn], F32, tag="rstd")
            nc.vector.reciprocal(rstd, sd)
            for j in range(KC):
                t1 = P['stat'].tile([128, cn], F32, tag="t1", bufs=2)
                nc.vector.tensor_sub(t1, src[:, j, c0:c0 + cn], ps_mu)
                if wnz:
                    nc.vector.scalar_tensor_tensor(
                        out=dst[:, j, c0:c0 + cn], in0=t1,
                        scalar=wt[:, j:j + 1], in1=rstd,
                        op0=ALU.mult, op1=ALU.mult)
                else:
                    nc.vector.tensor_mul(dst[:, j, c0:c0 + cn], t1, rstd)
                if bnz:
                    nc.vector.tensor_scalar_add(
                        dst[:, j, c0:c0 + cn], dst[:, j, c0:c0 + cn],
                        bt[:, j:j + 1])

        def proj_ws(wdram, l, n_mc, rhs_fn, epi_fn, kc=KC, wtag="wp6", pool='wp'):
            """Weight-stationary projection: out-chunk mc x accumulate kc."""
            for mc in range(n_mc):
                wt = P[pool].tile([128, kc, 128], F32R, tag=wtag)
                nc.sync.dma_start(out=wt, in_=wdram[l, mc].rearrange("p j m -> p j m"))
                rlists = [rhs_fn(0), rhs_fn(1)]
                pss = [P['ps1'].tile([128] + list(rlists[hf][0][1]), F32,
                                     tag="ps1", name=f"psw{mc}_{hf}")
                       for hf in range(2)]
                for j in range(kc):
                    for hf in range(2):
                        nc.tensor.matmul(pss[hf], wt[:, j, :], rlists[hf][j][0],
                                         start=(j == 0), stop=(j == kc - 1))
                for hf in range(2):
                    epi_fn(mc, hf, pss[hf])

        def attention(q_sb, k_sb, Vs, o_sb, col0, nq, mchunks):
            """One image's multi-head attention. Vs: list of (tile, m0, mlen).
            V tiles carry a ones column at [:, h, 64], so attn@v row 64 is the
            softmax denominator."""
            for hp2 in range(H // 2):
                pair = (2 * hp2, 2 * hp2 + 1)
                sts = {}
                # adjacent ST issue for row-group concurrency (heads at
                # partition 0-63 and 64-127)
                for h in pair:
                    hp, j = (h % 2) * 64, h // 2
                    sts[h] = []
                    for ci, (m0, mlen) in enumerate(mchunks):
                        ps_st = P['ps1'].tile([mlen, nq], F32, tag="ps1")
                        nc.tensor.matmul(
                            ps_st,
                            k_sb[hp:hp + 64, j, col0 + m0:col0 + m0 + mlen],
                            q_sb[hp:hp + 64, j, col0:col0 + nq],
                            start=True, stop=True)
                        sts[h].append(ps_st)
                pts = {}
                for h in pair:
                    pts[h] = []
                    for ci, (m0, mlen) in enumerate(mchunks):
                        pt = P['pt'].tile([mlen, nq], BF16, tag="pt")
                        nc.scalar.activation(out=pt, in_=sts[h][ci], func=AF.Exp)
                        pts[h].append((pt, m0, mlen))
                ps_os = {}
                for h in pair:
                    ps_o = P['ps1'].tile([65, nq], F32, tag="ps1")
                    for ci, (pt, m0, mlen) in enumerate(pts[h]):
                        nc.tensor.matmul(
                            ps_o, Vs[ci][0][:, h, :], pt,
                            start=(ci == 0), stop=(ci == len(pts[h]) - 1))
                    ps_os[h] = ps_o
                for h in pair:
                    ps_o = ps_os[h]
                    r_s = P['small'].tile([65, nq], BF16, tag="rs", bufs=4)
                    nc.scalar.activation(out=r_s[64:65, :], in_=ps_o[64:65, :],
                                         func=AF.Copy)
                    ps_R = P['ps1'].tile([64, nq], F32, tag="psR", bufs=1)
                    nc.tensor.matmul(ps_R, ones_r64[64:65, :],
                                     r_s[64:65, :], start=True, stop=True)
                    rinv = P['small'].tile([64, nq], F32, tag="rinv")
                    nc.vector.reciprocal(rinv, ps_R)
                    colp = (h % 2) * 64
                    nc.vector.tensor_mul(
                        o_sb[colp:colp + 64, h // 2, col0:col0 + nq],
                        ps_o[0:64, :], rinv)

        def vproj(src, wv_t, bias_nz, bias_t, col0, nt, vtag):
            """Activation-stationary V projection for one image -> token-major."""
            out = []
            mchunks = [(0, 128), (128, nt - 128)]
            for ci, (m0, mlen) in enumerate(mchunks):
                vt = P['v'].tile([mlen, 12, 65], BF16, tag=vtag, bufs=2)
                nc.vector.memset(vt.rearrange("p h e -> p (h e)"), 1.0)
                for hf in range(2):
                    ps = P['ps2'].tile([mlen, 384], F32, tag="ps2", bufs=2)
                    for j in range(KC):
                        nc.tensor.matmul(
                            ps,
                            src[:, j, col0 + m0:col0 + m0 + mlen],
                            wv_t[:, j, hf * 384:(hf + 1) * 384],
                            start=(j == 0), stop=(j == KC - 1))
                    sl = vt[:, hf * 6:(hf + 1) * 6, 0:64]
                    if bias_nz:
                        nc.vector.tensor_add(
                            sl, ps.rearrange("p (h e) -> p h e", e=64),
                            bias_t[:mlen, hf * 384:(hf + 1) * 384]
                            .rearrange("p (h e) -> p h e", e=64))
                    else:
                        nc.vector.tensor_copy(
                            sl, ps.rearrange("p (h e) -> p h e", e=64))
                out.append((vt, m0, mlen))
            return out

        # ---------------- transformer layers ------------------------------
        for l in range(L):
            # qk projection (weight-stationary, 12 out-chunks)
            q_sb = P['qk'].tile([128, KC, T], BF16, tag="q")
            k_sb = P['qk'].tile([128, KC, T], BF16, tag="k")
            qkb_t = None
            if flags['qkb']:
                qkb_t = P['small'].tile([128, 12], F32, tag="qkb")
                nc.sync.dma_start(out=qkb_t, in_=D['qkb'][l])

            def qk_epi(mc, half, ps):
                dst = (q_sb if mc < KC else k_sb)
                jj = mc % KC
                sl = dst[:, jj, half * HALF:(half + 1) * HALF]
                if mc < KC:
                    nc.scalar.activation(
                        out=sl, in_=ps, func=AF.Copy, scale=SCALE,
                        bias=(qkb_t[:, mc:mc + 1] if flags['qkb'] else 0.0))
                else:
                    if flags['qkb']:
                        nc.vector.tensor_scalar_add(sl, ps, qkb_t[:, mc:mc + 1])
                    else:
                        nc.vector.tensor_copy(sl, ps)

            proj_ws(D['wqk'], l, 12,
                    lambda half: [(y[:, j, half * HALF:(half + 1) * HALF], (HALF,))
                                  for j in range(KC)],
                    qk_epi)

            if l == 0:
                tap('q', q_sb, (128, KC, T)); tap('k', k_sb, (128, KC, T))

            # v projection per image
            wv_t = P['wv'].tile([128, KC, 768], F32R, tag="wv")
            nc.sync.dma_start(out=wv_t, in_=D['wv'][l])
            vb_t = None
            if flags['vb']:
                vb_t = P['small'].tile([128, 768], F32, tag="vb")
                nc.sync.dma_start(out=vb_t, in_=D['vb'][l])
            Vs = [vproj(y, wv_t, flags['vb'], vb_t, img * NP1, NP1, f"v{img}")
                  for img in range(BIMG)]

            # attention per image -> o_sb
            o_sb = P['scr'].tile([128, KC, T], F32R, tag="osb")
            for img in range(BIMG):
                attention(q_sb, k_sb, Vs[img], o_sb, img * NP1, NP1,
                          [(0, 128), (128, 69)])

            if l == 0:
                tap('osb', o_sb, (128, KC, T))
                for im in range(BIMG):
                    for ci, (vt, m0, mlen) in enumerate(Vs[im]):
                        tap(f'v{im}_{ci}', vt, (mlen, 768))

            # o projection + residual -> s1, then LN1 -> y1
            ob_t = None
            if flags['ob']:
                ob_t = P['small'].tile([128, KC], F32, tag="ob")
                nc.sync.dma_start(out=ob_t, in_=D['ob'][l])
            s1 = P['resid'].tile([128, KC, T], F32R, tag="scr1", bufs=1)

            def o_epi(mc, half, ps):
                sl = s1[:, mc, half * HALF:(half + 1) * HALF]
                nc.vector.tensor_add(sl, ps, y[:, mc, half * HALF:(half + 1) * HALF])
                if flags['ob']:
                    nc.vector.tensor_scalar_add(sl, sl, ob_t[:, mc:mc + 1])

            proj_ws(D['wo'], l, KC,
                    lambda half: [(o_sb[:, j, half * HALF:(half + 1) * HALF], (HALF,))
                                  for j in range(KC)],
                    o_epi)

            if l == 0:
                tap('s1', s1, (128, KC, T))
            ln1w = ln1b = None
            if flags['ln1_w']:
                ln1w = P['small'].tile([128, KC], F32, tag="lnw")
                nc.sync.dma_start(out=ln1w, in_=D['ln1_w'][l])
            if flags['ln1_b']:
                ln1b = P['small'].tile([128, KC], F32, tag="lnb")
                nc.sync.dma_start(out=ln1b, in_=D['ln1_b'][l])
            y1 = P['resid'].tile([128, KC, T], F32R, tag="resid")
            for half in range(2):
                layernorm(s1, y1, half * HALF, HALF, ln1w, ln1b,
                          flags['ln1_w'], flags['ln1_b'])

            if l == 0:
                tap('y1', y1, (128, KC, T))
            # FFN
            f1b_t = None
            if flags['f1b']:
                f1b_t = P['small'].tile([128, 24], F32, tag="f1b")
                nc.sync.dma_start(out=f1b_t, in_=D['f1b'][l])
            f2b_t = None
            if flags['f2b']:
                f2b_t = P['small'].tile([128, KC], F32, tag="f2b")
                nc.sync.dma_start(out=f2b_t, in_=D['f2b'][l])
            s2 = P['resid'].tile([128, KC, T], F32R, tag="scr1", bufs=1)
            for half in range(2):
                relus = []

                def f1_epi(mc, hf, ps, _relus=relus, _half=half):
                    rt = P['relu'].tile([128, HALF], F32R, tag="relu")
                    nc.scalar.activation(
                        out=rt, in_=ps, func=AF.Relu,
                        bias=(f1b_t[:, mc:mc + 1] if flags['f1b'] else 0.0))
                    _relus.append(rt)

                for mc in range(24):
                    wt = P['wp'].tile([128, KC, 128], F32R, tag="wp6")
                    nc.sync.dma_start(out=wt, in_=D['wf1'][l, mc])
                    ps = P['ps1'].tile([128, HALF], F32, tag="ps1")
                    for j in range(KC):
                        nc.tensor.matmul(
                            ps, wt[:, j, :],
                            y1[:, j, half * HALF:(half + 1) * HALF],
                            start=(j == 0), stop=(j == KC - 1))
                    f1_epi(mc, half, ps)
                for mc in range(KC):
                    ps = P['ps1'].tile([128, HALF], F32, tag="ps1")
                    for kh in range(2):
                        wt = P['w24'].tile([128, 12, 128], F32R, tag="wp24")
                        nc.sync.dma_start(
                            out=wt, in_=D['wf2'][l, mc, :, kh * 12:(kh + 1) * 12, :])
                        for jj in range(12):
                            j = kh * 12 + jj
                            nc.tensor.matmul(ps, wt[:, jj, :], relus[j],
                                             start=(j == 0), stop=(j == 23))
                    sl = s2[:, mc, half * HALF:(half + 1) * HALF]
                    nc.vector.tensor_add(
                        sl, ps, y1[:, mc, half * HALF:(half + 1) * HALF])
                    if flags['f2b']:
                        nc.vector.tensor_scalar_add(sl, sl, f2b_t[:, mc:mc + 1])

            if l == 0:
                tap('s2', s2, (128, KC, T))
            ln2w = ln2b = None
            if flags['ln2_w']:
                ln2w = P['small'].tile([128, KC], F32, tag="lnw")
                nc.sync.dma_start(out=ln2w, in_=D['ln2_w'][l])
            if flags['ln2_b']:
                ln2b = P['small'].tile([128, KC], F32, tag="lnb")
                nc.sync.dma_start(out=ln2b, in_=D['ln2_b'][l])
            y2 = P['resid'].tile([128, KC, T], F32R, tag="resid")
            for half in range(2):
                layernorm(s2, y2, half * HALF, HALF, ln2w, ln2b,
                          flags['ln2_w'], flags['ln2_b'])

            if l == 0:
                tap('y2', y2, (128, KC, T))
            # ---- logo attention on patch tokens (no residual, cls kept) --
            lq_sb = P['qk'].tile([128, KC, T], BF16, tag="q")
            lk_sb = P['qk'].tile([128, KC, T], BF16, tag="k")

            def patch_rhs(src, j, half):
                return src[:, j, :].rearrange("p (i n) -> p i n", n=NP1)[
                    :, half * 2:(half + 1) * 2, 1:197]

            def lqk_epi(mc, half, ps):
                dst = (lq_sb if mc < KC else lk_sb)
                jj = mc % KC
                sl = patch_rhs(dst, jj, half)
                if mc < KC:
                    nc.scalar.activation(out=sl, in_=ps, func=AF.Copy, scale=SCALE)
                else:
                    nc.vector.tensor_copy(sl, ps)

            proj_ws(D['wlqk'], l, 12,
                    lambda half: [(patch_rhs(y2, j, half), (2, 196))
                                  for j in range(KC)],
                    lqk_epi)

            wlv_t = P['wv'].tile([128, KC, 768], F32R, tag="wv")
            nc.sync.dma_start(out=wlv_t, in_=D['wlv'][l])
            LVs = [vproj(y2, wlv_t, False, None, img * NP1 + 1, 196, f"v{img}")
                   for img in range(BIMG)]

            lo_sb = P['scr'].tile([128, KC, T], F32R, tag="osb")
            for img in range(BIMG):
                attention(lq_sb, lk_sb, LVs[img], lo_sb, img * NP1 + 1, 196,
                          [(0, 128), (128, 68)])

            lpb_t = None
            if flags['lpb']:
                lpb_t = P['small'].tile([128, KC], F32, tag="ob")
                nc.sync.dma_start(out=lpb_t, in_=D['lpb'][l])

            def lp_epi(mc, half, ps):
                sl = patch_rhs(y2, mc, half)
                if flags['lpb']:
                    nc.vector.tensor_scalar_add(sl, ps, lpb_t[:, mc:mc + 1])
                else:
                    nc.vector.tensor_copy(sl, ps)

            proj_ws(D['wlp'], l, KC,
                    lambda half: [(patch_rhs(lo_sb, j, half), (2, 196))
                                  for j in range(KC)],
                    lp_epi)

            y = y2
            if debug_taps:
                nc.sync.dma_start(out=dbg_d[l + 1], in_=y)

        # ---------------- final LN + classifier ---------------------------
        cls_ap = y.rearrange("p j (i n) -> p j i n", n=NP1)[:, :, :, 0]  # [128,6,4]
        normw_t = P['small'].tile([128, KC], F32, tag="lnw")
        nc.sync.dma_start(out=normw_t, in_=D['normw'][:, :])
        normb_t = P['small'].tile([128, KC], F32, tag="lnb")
        nc.sync.dma_start(out=normb_t, in_=D['normb'][:, :])
        yf = P['small'].tile([128, KC, BIMG], F32R, tag="yf")
        # LN over E on 4 cls columns
        xsq = P['stat'].tile([128, KC, BIMG], F32R, tag="fsq")
        for j in range(KC):
            nc.vector.tensor_mul(xsq[:, j, :], cls_ap[:, j, :], cls_ap[:, j, :])
        ps_mu = P['ps1'].tile([128, BIMG], F32, tag="ps1")
        for j in range(KC):
            nc.tensor.matmul(ps_mu, inv768, cls_ap[:, j, :],
                             start=(j == 0), stop=(j == KC - 1))
        ps_m2 = P['ps1'].tile([128, BIMG], F32, tag="ps1")
        for j in range(KC):
            nc.tensor.matmul(ps_m2, inv768, xsq[:, j, :],
                             start=(j == 0), stop=(j == KC - 1))
        musq = P['stat'].tile([128, BIMG], F32, tag="fmusq")
        nc.scalar.activation(out=musq, in_=ps_mu, func=AF.Square)
        var = P['stat'].tile([128, BIMG], F32, tag="fvar")
        nc.vector.tensor_sub(var, ps_m2, musq)
        sd = P['stat'].tile([128, BIMG], F32, tag="fsd")
        nc.scalar.activation(out=sd, in_=var, func=AF.Sqrt, bias=eps_t)
        rstd = P['stat'].tile([128, BIMG], F32, tag="frstd")
        nc.vector.reciprocal(rstd, sd)
        for j in range(KC):
            t1 = P['stat'].tile([128, BIMG], F32, tag="ft1")
            nc.vector.tensor_sub(t1, cls_ap[:, j, :], ps_mu)
            if flags['normw']:
                nc.vector.scalar_tensor_tensor(
                    out=yf[:, j, :], in0=t1, scalar=normw_t[:, j:j + 1],
                    in1=rstd, op0=ALU.mult, op1=ALU.mult)
            else:
                nc.vector.tensor_mul(yf[:, j, :], t1, rstd)
            if flags['normb']:
                nc.vector.tensor_scalar_add(yf[:, j, :], yf[:, j, :],
                                            normb_t[:, j:j + 1])
        clsw_t = P['small'].tile([128, KC, 4], F32R, tag="clsw")
        nc.sync.dma_start(out=clsw_t, in_=D['clsw'][:, :, :])
        ps_c = P['ps1'].tile([BIMG, 4], F32, tag="ps1")
        for j in range(KC):
            nc.tensor.matmul(ps_c, yf[:, j, :], clsw_t[:, j, :],
                             start=(j == 0), stop=(j == KC - 1))
        out_sb = P['small'].tile([BIMG, 4], F32, tag="outsb")
        if flags['clsb']:
            clsb_t = P['small'].tile([BIMG, 4], F32, tag="clsb")
            nc.sync.dma_start(out=clsb_t, in_=D['clsb'][:, :])
            nc.vector.tensor_add(out_sb, ps_c, clsb_t)
        else:
            nc.vector.tensor_copy(out_sb, ps_c)
        nc.sync.dma_start(out=out_d[:, :], in_=out_sb)

    _fix_waits(nc)
    nc._tap_names = list(taps)
    return nc


# -------------------------------------------------------------------- entry

_BUILD_CACHE = {}


def kernel(**inputs):
    d, flags, L = _host_prep(inputs)
    key = (L, tuple(sorted(flags.items())))
    debug_taps = bool(globals().get('DEBUG_TAPS'))
    key = key + (debug_taps,)
    if key not in _BUILD_CACHE:
        _BUILD_CACHE[key] = build(L, flags, debug_taps=debug_taps)
    nc = _BUILD_CACHE[key]
    x = np.asarray(inputs['x'], np.float32)
    in_maps = []
    for core in range(NCORES):
        m = dict(d)
        m['x'] = np.ascontiguousarray(x[core * BIMG:(core + 1) * BIMG])
        in_maps.append(m)
    br = run_bass_kernel_spmd(nc, in_maps, list(range(NCORES)))
    out = np.concatenate([br.results[i]['out'] for i in range(NCORES)], axis=0)
    if debug_taps:
        globals()['LAST_DBG'] = [br.results[i]['dbg'] for i in range(NCORES)]
        globals()['LAST_TAPS'] = [
            {k: br.results[i]['tap_' + k] for k in nc._tap_names}
            for i in range(NCORES)]
    return out
